# revision 1
# baseline (speedup 1.0000x reference)
"""Trainium2 Bass kernel for nn_Conv_agg (edge-parallel GNN message passing).

Math (see reference):
    out[n] = sum_k ( sum_{e: src(e)=n} X[e,k] * h[tgt(e)] ) @ W[k] + bias

Structure exploited (asserted at runtime, guaranteed by setup_inputs):
  - src(e) = e // DEG exactly (each node emits DEG=16 consecutive edges)
  - edges/nodes of graph g are contiguous and tgt(e) stays inside graph g's
    node window -> sharding 125 graphs per core is a perfect partition
    (no cross-core edges, no collectives).

Per-core device pipeline (chunks of 4096 edges = 256 nodes = 32 PE blocks):
  1. dma_gather: G[128 edge-part, 32, 128] = h[tgt] rows (512B each) from HBM
  2. DVE: Xall[p, b, j, k] = Xr[p, b, k] * blockdiag_mask[p, j, k]
  3. PE stage 1 (per 128-edge block b): A^T slice [128 cin, 16] =
         G_b[128e, 128c].T @ Xall_b[128e, 16]   (weighted 16-edge segment sum)
  4. copy PSUM A^T -> SBUF
  5. PE stage 2 (per 128-node unit, accumulate k=0,1):
         out[128 nodes, 128 cout] += (A_k^T).T @ W_k
  6. DVE adds bias, DMA out rows.
"""

import numpy as np

B, NPG, DEG, K, CIN, COUT = 1000, 100, 16, 2, 128, 128
E = B * NPG * DEG            # 1,600,000 edges
NT = B * NPG                 # 100,000 nodes
NCORES = 8
NT_C = NT // NCORES          # 12,500 nodes / core
E_C = E // NCORES            # 200,000 edges / core
EB = 32                      # 128-edge blocks per chunk
CHUNK_E = EB * 128           # 4096 edges / chunk
CHUNK_N = CHUNK_E // DEG     # 256 nodes / chunk
N_CHUNKS = -(-E_C // CHUNK_E)   # 49
E_PAD = N_CHUNKS * CHUNK_E   # 200,704
UNITS = CHUNK_N // 128       # stage-2 units of 128 nodes per chunk

_module_cache = {}


def _patch_tile_drain():
    """This walrus build allows a single sync-wait per instruction; Tile's
    kernel-tail drain aggregates one wait per outstanding sem onto one
    InstDrain. Hoist extras onto dedicated sync nops (sequential on SP)."""
    import concourse.mybir as mybir
    from concourse.tile import TileContext
    from concourse.vector_clock import ScopedClock

    if getattr(TileContext, "_drain_patched", False):
        return

    def _drain_and_barrier(self, tick_clock, wait_clock):
        probe = self.nc.sync.nop(nofuse=True)
        wait_clock.add_sem_waits(probe.ins, ScopedClock({None: tick_clock.global_clock}))
        si = probe.ins.sync_info
        waits = list(si.on_wait) if si is not None and si.on_wait else []
        if si is not None and len(waits) > 1:
            si.on_wait = waits[:1]
            for w in waits[1:]:
                n = self.nc.sync.nop(nofuse=True)
                n.ins.sync_info = mybir.SyncInfo(on_wait=[w], on_update=[])
        self.nc.sync.drain()
        self.nc.all_engine_barrier()
        assert self.sems is not None
        popped = self.nc._tile_sem_poison_stack.pop()
        assert popped is self._sem_poison
        self.nc.clear_and_free_semaphores(list(self.sems.allocated().values()))
        self.nc.all_engine_barrier()

    TileContext._drain_and_barrier = _drain_and_barrier
    TileContext._drain_patched = True


def _build_module():
    import concourse.bacc as bacc
    import concourse.mybir as mybir
    from concourse.tile import TileContext

    _patch_tile_drain()
    f32 = mybir.dt.float32

    nc = bacc.Bacc("TRN2", target_bir_lowering=False)
    h_t = nc.dram_tensor("h", [NT_C, CIN], f32, kind="ExternalInput")
    idx_t = nc.dram_tensor("idx", [N_CHUNKS, 128, CHUNK_E // 16], mybir.dt.int16,
                           kind="ExternalInput")
    xr_t = nc.dram_tensor("xr", [N_CHUNKS, 128, EB * K], f32, kind="ExternalInput")
    mask_t = nc.dram_tensor("mask", [128, 8 * K], f32, kind="ExternalInput")
    w_t = nc.dram_tensor("w", [K, CIN, COUT], f32, kind="ExternalInput")
    bias_t = nc.dram_tensor("bias", [128, COUT], f32, kind="ExternalInput")
    out_t = nc.dram_tensor("out", [NT_C, COUT], f32, kind="ExternalOutput")

    with TileContext(nc) as tc:
        with (
            tc.tile_pool(name="consts", bufs=1) as cpool,
            tc.tile_pool(name="idxp", bufs=3) as idxpool,
            tc.tile_pool(name="xrp", bufs=3) as xrpool,
            tc.tile_pool(name="gp", bufs=3) as gpool,
            tc.tile_pool(name="xap", bufs=3) as xapool,
            tc.tile_pool(name="atp", bufs=3) as atpool,
            tc.tile_pool(name="outp", bufs=3) as outpool,
            tc.tile_pool(name="psA", bufs=2, space="PSUM") as psumA,
            tc.tile_pool(name="psO", bufs=2, space="PSUM") as psumO,
        ):
            mask_sb = cpool.tile([128, 8, K], f32)
            nc.sync.dma_start(mask_sb[:, :, :],
                              mask_t[:, :].rearrange("p (j k) -> p j k", k=K))
            w_sb = cpool.tile([128, K, COUT], f32)
            nc.sync.dma_start(w_sb[:, :, :], w_t[:, :, :].rearrange("k c o -> c k o"))
            bias_sb = cpool.tile([128, COUT], f32)
            nc.sync.dma_start(bias_sb[:, :], bias_t[:, :])

            for c in range(N_CHUNKS):
                idx_sb = idxpool.tile([128, CHUNK_E // 16], mybir.dt.int16)
                nc.sync.dma_start(idx_sb[:, :], idx_t[c, :, :])
                xr_sb = xrpool.tile([128, EB, K], f32)
                nc.sync.dma_start(xr_sb[:, :, :],
                                  xr_t[c, :, :].rearrange("p (b k) -> p b k", k=K))

                # HW limit: 1024 idxs (64 desc/engine packet) per dma_gather
                g_sb = gpool.tile([128, EB, CIN], f32)
                for i in range(CHUNK_E // 1024):
                    nc.gpsimd.dma_gather(
                        out_ap=g_sb[:, 8 * i:8 * (i + 1), :],
                        in_ap=h_t[:, :],
                        idxs_ap=idx_sb[:, 64 * i:64 * (i + 1)],
                        num_idxs=1024,
                        num_idxs_reg=1024,
                        elem_size=CIN,
                    )

                # Xall[p, b, j, k] = Xr[p, b, k] * mask[p, j, k]
                xall = xapool.tile([128, EB, 8, K], f32)
                nc.vector.tensor_tensor(
                    xall[:, :, :, :],
                    xr_sb[:, :, :].unsqueeze(2).broadcast_to([128, EB, 8, K]),
                    mask_sb[:, :, :].unsqueeze(1).broadcast_to([128, EB, 8, K]),
                    op=mybir.AluOpType.mult,
                )

                # stage 1: A^T chunk [128 cin, EB*16] in one PSUM bank
                psA_tl = psumA.tile([128, EB, 16], f32)
                for b in range(EB):
                    nc.tensor.matmul(
                        psA_tl[:, b, :],
                        g_sb[:, b, :],          # lhsT [128 edges, 128 cin]
                        xall[:, b, :, :],       # rhs  [128 edges, 16]
                        start=True, stop=True,
                    )
                aT_sb = atpool.tile([128, EB * 16], f32)
                nc.any.tensor_copy(aT_sb[:, :],
                                   psA_tl[:, :, :].rearrange("p b j -> p (b j)"))

                # stage 2 + bias + store, per 128-node unit
                aT_v = aT_sb[:, :].rearrange("p (m k) -> p m k", k=K)
                for u in range(UNITS):
                    psO_tl = psumO.tile([128, COUT], f32)
                    for k in range(K):
                        nc.tensor.matmul(
                            psO_tl[:, :],
                            aT_v[:, u * 128:(u + 1) * 128, k],  # lhsT [cin, nodes]
                            w_sb[:, k, :],                       # rhs  [cin, cout]
                            start=(k == 0), stop=(k == K - 1),
                        )
                    o_sb = outpool.tile([128, COUT], f32)
                    nc.vector.tensor_tensor(o_sb[:, :], psO_tl[:, :], bias_sb[:, :],
                                            op=mybir.AluOpType.add)
                    n0 = c * CHUNK_N + u * 128
                    rows = min(128, NT_C - n0)
                    if rows > 0:
                        nc.sync.dma_start(out_t[n0:n0 + rows, :], o_sb[:rows, :])
    nc.compile()
    return nc


def _get_module():
    if "nc" not in _module_cache:
        _module_cache["nc"] = _build_module()
    return _module_cache["nc"]


def _prep_inputs(h, X, tgt, weight, bias):
    """Host-side sharding/layout (no arithmetic on data values)."""
    # per-core local target ids, padded to E_PAD with 0 (masked by X pad = 0)
    tgt_loc = (tgt.reshape(NCORES, E_C)
               - (np.arange(NCORES, dtype=np.int64) * NT_C)[:, None])
    assert tgt_loc.min() >= 0 and tgt_loc.max() < NT_C, "tgt escapes core block"
    idxp = np.zeros((NCORES, E_PAD), np.int16)
    idxp[:, :E_C] = tgt_loc.astype(np.int16)
    # per 1024-idx sub-gather i: idx j -> partition j%16, col 64*i + j//16;
    # replicate the 16-partition block x8
    idx_arr = idxp.reshape(NCORES, N_CHUNKS, CHUNK_E // 1024, 64, 16)
    idx_arr = idx_arr.transpose(0, 1, 4, 2, 3).reshape(NCORES, N_CHUNKS, 16, CHUNK_E // 16)
    idx_arr = np.ascontiguousarray(np.tile(idx_arr, (1, 1, 8, 1)))

    xp = np.zeros((NCORES, E_PAD, K), np.float32)
    xp[:, :E_C] = X.reshape(NCORES, E_C, K)
    # xr[c, ch, p, b, k] = X[base + 128*b + p, k]
    xr = xp.reshape(NCORES, N_CHUNKS, EB, 128, K).transpose(0, 1, 3, 2, 4)
    xr = np.ascontiguousarray(xr).reshape(NCORES, N_CHUNKS, 128, EB * K)

    mask = np.zeros((128, 8, K), np.float32)
    for p in range(128):
        mask[p, p // 16, :] = 1.0
    mask = mask.reshape(128, 8 * K)

    bias_rep = np.ascontiguousarray(np.broadcast_to(bias, (128, COUT))).astype(np.float32)
    return idx_arr, xr, mask, weight.astype(np.float32, copy=False), bias_rep


def kernel(h, X, edge_index, node_index, batch_node, batch_edge, num_node,
           weight, bias):
    from concourse.bass_utils import run_bass_kernel_spmd

    h = np.asarray(h, np.float32)
    X = np.asarray(X, np.float32)
    edge_index = np.asarray(edge_index)
    weight = np.asarray(weight, np.float32)
    bias = np.asarray(bias, np.float32)

    src = np.asarray(edge_index[1])
    tgt = np.asarray(edge_index[2])
    # structural contract from setup_inputs (see module docstring)
    assert src.shape == (E,) and h.shape == (NT, CIN) and X.shape == (E, K)
    assert np.array_equal(src, np.arange(E, dtype=src.dtype) // DEG), \
        "edges not sorted as src=e//DEG"

    idx_arr, xr, mask, w, bias_rep = _prep_inputs(h, X, tgt, weight, bias)

    nc = _get_module()
    in_maps = []
    for c in range(NCORES):
        in_maps.append({
            "h": np.ascontiguousarray(h[c * NT_C:(c + 1) * NT_C]),
            "idx": idx_arr[c],
            "xr": xr[c],
            "mask": mask,
            "w": w,
            "bias": bias_rep,
        })
    res = run_bass_kernel_spmd(nc, in_maps, core_ids=list(range(NCORES)))
    out = np.concatenate([r["out"] for r in res.results], axis=0)
    return out



# revision 5
# speedup vs baseline: 4.8353x; 4.8353x over previous
"""Trainium2 Bass kernel for nn_Conv_agg (edge-parallel GNN message passing).

Math (see reference):
    out[n] = sum_k ( sum_{e: src(e)=n} X[e,k] * h[tgt(e)] ) @ W[k] + bias

Structure exploited (asserted at runtime, guaranteed by setup_inputs):
  - src(e) = e // DEG exactly (each node emits DEG=16 consecutive edges)
  - edges/nodes of graph g are contiguous and tgt(e) stays inside graph g's
    100-node window -> 125 graphs per core is a perfect partition
    (no cross-core edges, no collectives).

Per-graph dense-scatter formulation (no per-edge DMA gather: the baseline's
gpsimd SWDGE descriptor generation was 98% of exec time):
    S_k[n, m] = sum_{e: src=n, tgt=m} X[e, k]     (100x100 per graph, k=0,1)
    out_g     = sum_k (S_k @ h_g) @ W_k + bias

Device pipeline per graph (1600 edges padded to 13 blocks of 128):
  1. DVE one-hot: O[e, b, m] = (tgt[e,b] == iota[m])          [128, 13, 128] bf16
  2. DVE:        Xall[e, b, k, s] = X[e, b, k] * (e//16 == s) [128, 13, 2, 8]
  3. PE S-build: psS[m, (b,k,s)] += O_b^T @ Xall_b  -> S^T    [128, 208] (13 mm)
  4. PE stage A: psMT = h_g^T-free @ S^T   (lhsT=h_g [m,c])   [128 c, 208]
  5. PE stage B: psO[n, o] = sum_k mT_k^T @ W_k               [104, 128]
  6. DVE bias add from PSUM -> staging; batched DMA out (5 graphs).

All matmul operands bf16 (FWL weight loads), PSUM f32, output f32.
"""

import numpy as np

B, NPG, DEG, K, CIN, COUT = 1000, 100, 16, 2, 128, 128
E = B * NPG * DEG            # 1,600,000 edges
NT = B * NPG                 # 100,000 nodes
NCORES = 8
G_C = B // NCORES            # 125 graphs / core
NT_C = NT // NCORES          # 12,500 nodes / core
E_C = E // NCORES            # 200,000 edges / core
EG = NPG * DEG               # 1600 edges / graph
BLK = 13                     # 128-edge blocks per graph (1664 padded)
EGP = BLK * 128              # 1664 padded edges / graph
NSEG = BLK * 8               # 104 src slots per graph (100 real)
SCOL = BLK * K * 8           # 208 S^T columns (b, k, s)
OUTB = 5                     # graphs per output DMA batch

_module_cache = {}


def _patch_tile_drain():
    """This walrus build allows a single sync-wait per instruction; Tile's
    kernel-tail drain aggregates one wait per outstanding sem onto one
    InstDrain. Hoist extras onto dedicated sync nops (sequential on SP)."""
    import concourse.mybir as mybir
    from concourse.tile import TileContext
    from concourse.vector_clock import ScopedClock

    if getattr(TileContext, "_drain_patched", False):
        return

    def _drain_and_barrier(self, tick_clock, wait_clock):
        probe = self.nc.sync.nop(nofuse=True)
        wait_clock.add_sem_waits(probe.ins, ScopedClock({None: tick_clock.global_clock}))
        si = probe.ins.sync_info
        waits = list(si.on_wait) if si is not None and si.on_wait else []
        if si is not None and len(waits) > 1:
            si.on_wait = waits[:1]
            for w in waits[1:]:
                n = self.nc.sync.nop(nofuse=True)
                n.ins.sync_info = mybir.SyncInfo(on_wait=[w], on_update=[])
        self.nc.sync.drain()
        self.nc.all_engine_barrier()
        assert self.sems is not None
        popped = self.nc._tile_sem_poison_stack.pop()
        assert popped is self._sem_poison
        self.nc.clear_and_free_semaphores(list(self.sems.allocated().values()))
        self.nc.all_engine_barrier()

    TileContext._drain_and_barrier = _drain_and_barrier
    TileContext._drain_patched = True


def _build_module():
    import concourse.bacc as bacc
    import concourse.mybir as mybir
    from concourse.tile import TileContext

    _patch_tile_drain()
    f32 = mybir.dt.float32
    bf16 = mybir.dt.bfloat16

    nc = bacc.Bacc("TRN2", target_bir_lowering=False)
    # all inputs partition-major from host
    h_t = nc.dram_tensor("h", [128, G_C * CIN], bf16, kind="ExternalInput")
    tgt_t = nc.dram_tensor("tgt", [128, G_C * BLK], bf16, kind="ExternalInput")
    xr_t = nc.dram_tensor("xr", [128, G_C * BLK * K], bf16, kind="ExternalInput")
    iota_t = nc.dram_tensor("iota", [128, 128], bf16, kind="ExternalInput")
    mask_t = nc.dram_tensor("mask", [128, 8], bf16, kind="ExternalInput")
    w_t = nc.dram_tensor("w", [128, K * COUT], bf16, kind="ExternalInput")
    bias_t = nc.dram_tensor("bias", [128, COUT], f32, kind="ExternalInput")
    out_t = nc.dram_tensor("out", [NT_C, COUT], f32, kind="ExternalOutput")

    with TileContext(nc) as tc:
        with (
            tc.tile_pool(name="consts", bufs=1) as cpool,
            tc.tile_pool(name="op", bufs=2) as opool,
            tc.tile_pool(name="xap", bufs=2) as xapool,
            tc.tile_pool(name="stp", bufs=2) as stpool,
            tc.tile_pool(name="mtp", bufs=2) as mtpool,
            tc.tile_pool(name="outp", bufs=2) as outpool,
            tc.tile_pool(name="psS", bufs=2, space="PSUM") as psumS,
            tc.tile_pool(name="psMT", bufs=2, space="PSUM") as psumMT,
            tc.tile_pool(name="psO", bufs=2, space="PSUM") as psumO,
        ):
            h_sb = cpool.tile([128, G_C, CIN], bf16)
            nc.sync.dma_start(h_sb[:, :, :],
                              h_t[:, :].rearrange("p (g c) -> p g c", c=CIN))
            tgt_sb = cpool.tile([128, G_C, BLK], bf16)
            nc.sync.dma_start(tgt_sb[:, :, :],
                              tgt_t[:, :].rearrange("p (g b) -> p g b", b=BLK))
            xr_sb = cpool.tile([128, G_C, BLK, K], bf16)
            nc.sync.dma_start(xr_sb[:, :, :, :],
                              xr_t[:, :].rearrange("p (g b k) -> p g b k",
                                                   b=BLK, k=K))
            iota_sb = cpool.tile([128, 128], bf16)
            nc.sync.dma_start(iota_sb[:, :], iota_t[:, :])
            mask_sb = cpool.tile([128, 8], bf16)
            nc.sync.dma_start(mask_sb[:, :], mask_t[:, :])
            w_sb = cpool.tile([128, K, COUT], bf16)
            nc.sync.dma_start(w_sb[:, :, :],
                              w_t[:, :].rearrange("p (k o) -> p k o", o=COUT))
            bias_sb = cpool.tile([128, COUT], f32)
            nc.sync.dma_start(bias_sb[:, :], bias_t[:, :])

            for g in range(G_C):
                # one-hot of local tgt: O[e, b, m] = (tgt == m)
                o_tl = opool.tile([128, BLK, 128], bf16)
                nc.vector.tensor_tensor(
                    o_tl[:, :, :],
                    iota_sb[:, :].unsqueeze(1).broadcast_to([128, BLK, 128]),
                    tgt_sb[:, g, :].unsqueeze(2).broadcast_to([128, BLK, 128]),
                    op=mybir.AluOpType.is_equal,
                )
                # block-diag X: Xall[e, b, k, s] = X[e, b, k] * (e//16 == s)
                xall_tl = xapool.tile([128, BLK, K * 8], bf16)
                xall_v = xall_tl[:, :, :].rearrange("p b (k s) -> p b k s", s=8)
                nc.vector.tensor_tensor(
                    xall_v,
                    xr_sb[:, g, :, :].unsqueeze(3).broadcast_to([128, BLK, K, 8]),
                    mask_sb[:, :].unsqueeze(1).unsqueeze(1)
                    .broadcast_to([128, BLK, K, 8]),
                    op=mybir.AluOpType.mult,
                )

                # S^T build: psS[m, (b, k, s)] = O_b^T @ Xall_b
                psS_tl = psumS.tile([128, BLK, K * 8], f32)
                for b in range(BLK):
                    nc.tensor.matmul(
                        psS_tl[:, b, :],
                        o_tl[:, b, :],          # lhsT [128 e, 128 m] (FWL)
                        xall_tl[:, b, :],       # rhs  [128 e, 16]
                        start=True, stop=True,
                    )
                sT_tl = stpool.tile([128, SCOL], bf16)
                nc.any.tensor_copy(sT_tl[:, :],
                                   psS_tl[:, :, :].rearrange("p b j -> p (b j)"))

                # stage A: psMT[c, (b,k,s)] = sum_m h[m, c] * S^T[m, (b,k,s)]
                psMT_tl = psumMT.tile([128, SCOL], f32)
                nc.tensor.matmul(
                    psMT_tl[:, :],
                    h_sb[:, g, :],              # lhsT [128 m, 128 c] (FWL)
                    sT_tl[:, :],                # rhs  [128 m, 208]
                    start=True, stop=True,
                )
                # copy reorders (b, k, s) -> (k, b, s) so each mT_k slice is
                # contiguous (matmul weight APs allow only one free dim)
                mT_tl = mtpool.tile([128, SCOL], bf16)
                nc.any.tensor_copy(
                    mT_tl[:, :].rearrange("p (k b s) -> p b k s", b=BLK, s=8),
                    psMT_tl[:, :].rearrange("p (b k s) -> p b k s", k=K, s=8))

                # stage B: psO[n, o] = sum_k mT_k^T @ W_k
                psO_tl = psumO.tile([128, COUT], f32)
                for k in range(K):
                    nc.tensor.matmul(
                        psO_tl[0:NSEG, :],
                        mT_tl[:, k * NSEG:(k + 1) * NSEG],  # lhsT [128 c, 104 n]
                        w_sb[:, k, :],          # rhs  [128 c, 128 o]
                        start=(k == 0), stop=(k == K - 1),
                    )

                # bias add + staged output (OUTB graphs per DMA)
                ob = g % OUTB
                if ob == 0:
                    out_tl = outpool.tile([128, OUTB, COUT], f32)
                nc.vector.tensor_tensor(
                    out_tl[0:NPG, ob, :], psO_tl[0:NPG, :], bias_sb[0:NPG, :],
                    op=mybir.AluOpType.add,
                )
                if ob == OUTB - 1:
                    g0 = g - (OUTB - 1)
                    dst = out_t[g0 * NPG:(g + 1) * NPG, :].rearrange(
                        "(gi m) o -> m gi o", gi=OUTB)
                    nc.sync.dma_start(dst, out_tl[0:NPG, :, :])
    nc.compile()
    return nc


def _get_module():
    if "nc" not in _module_cache:
        _module_cache["nc"] = _build_module()
    return _module_cache["nc"]


def _prep_inputs(h, X, tgt, weight, bias):
    """Host-side sharding/layout (indexing + dtype formatting only)."""
    import ml_dtypes
    bf16 = ml_dtypes.bfloat16

    # local tgt ids within each graph's 100-node window
    g_edge = np.arange(E, dtype=np.int64) // EG
    tgt_loc = tgt - g_edge * NPG
    assert tgt_loc.min() >= 0 and tgt_loc.max() < NPG, "tgt escapes graph block"

    # h: [core][p, g, c] bf16, graph rows padded 100 -> 128 (pad rows unused:
    # S^T rows 100..127 are zero because tgt < 100)
    h_pad = np.zeros((NCORES, G_C, 128, CIN), np.float32)
    h_pad[:, :, :NPG, :] = h.reshape(NCORES, G_C, NPG, CIN)
    h_plc = np.ascontiguousarray(h_pad.transpose(0, 2, 1, 3)).reshape(
        NCORES, 128, G_C * CIN).astype(bf16)

    # per-graph edge stream padded 1600 -> 1664 (pad X = 0 kills contribution)
    tgt_p = np.zeros((NCORES, G_C, EGP), np.float32)
    tgt_p[:, :, :EG] = tgt_loc.reshape(NCORES, G_C, EG)
    # [core, g, b, p] -> [core, p, g, b]
    tgt_plc = np.ascontiguousarray(
        tgt_p.reshape(NCORES, G_C, BLK, 128).transpose(0, 3, 1, 2)).reshape(
        NCORES, 128, G_C * BLK).astype(bf16)

    x_p = np.zeros((NCORES, G_C, EGP, K), np.float32)
    x_p[:, :, :EG, :] = X.reshape(NCORES, G_C, EG, K)
    # [core, g, b, p, k] -> [core, p, g, b, k]
    xr_plc = np.ascontiguousarray(
        x_p.reshape(NCORES, G_C, BLK, 128, K).transpose(0, 3, 1, 2, 4)).reshape(
        NCORES, 128, G_C * BLK * K).astype(bf16)

    iota = np.broadcast_to(np.arange(128, dtype=np.float32), (128, 128))
    iota = np.ascontiguousarray(iota).astype(bf16)
    mask = (np.arange(128)[:, None] // 16 == np.arange(8)[None, :])
    mask = mask.astype(np.float32).astype(bf16)

    # W: [c, k, o] bf16 partition-major
    w_plc = np.ascontiguousarray(weight.transpose(1, 0, 2)).reshape(
        128, K * COUT).astype(bf16)
    bias_rep = np.ascontiguousarray(
        np.broadcast_to(bias, (128, COUT))).astype(np.float32)
    return h_plc, tgt_plc, xr_plc, iota, mask, w_plc, bias_rep


def kernel(h, X, edge_index, node_index, batch_node, batch_edge, num_node,
           weight, bias):
    from concourse.bass_utils import run_bass_kernel_spmd

    h = np.asarray(h, np.float32)
    X = np.asarray(X, np.float32)
    edge_index = np.asarray(edge_index)
    weight = np.asarray(weight, np.float32)
    bias = np.asarray(bias, np.float32)

    src = np.asarray(edge_index[1])
    tgt = np.asarray(edge_index[2])
    # structural contract from setup_inputs (see module docstring)
    assert src.shape == (E,) and h.shape == (NT, CIN) and X.shape == (E, K)
    assert np.array_equal(src, np.arange(E, dtype=src.dtype) // DEG), \
        "edges not sorted as src=e//DEG"

    h_plc, tgt_plc, xr_plc, iota, mask, w_plc, bias_rep = _prep_inputs(
        h, X, tgt, weight, bias)

    nc = _get_module()
    in_maps = []
    for c in range(NCORES):
        in_maps.append({
            "h": h_plc[c],
            "tgt": tgt_plc[c],
            "xr": xr_plc[c],
            "iota": iota,
            "mask": mask,
            "w": w_plc,
            "bias": bias_rep,
        })
    res = run_bass_kernel_spmd(nc, in_maps, core_ids=list(range(NCORES)))
    out = np.concatenate([r["out"] for r in res.results], axis=0)
    return out


# revision 11
# speedup vs baseline: 4.9383x; 1.0213x over previous
"""Trainium2 Bass kernel for nn_Conv_agg (edge-parallel GNN message passing).

Math (see reference):
    out[n] = sum_k ( sum_{e: src(e)=n} X[e,k] * h[tgt(e)] ) @ W[k] + bias

Structure exploited (asserted at runtime, guaranteed by setup_inputs):
  - src(e) = e // DEG exactly (each node emits DEG=16 consecutive edges)
  - edges/nodes of graph g are contiguous and tgt(e) stays inside graph g's
    100-node window -> 125 graphs per core is a perfect partition
    (no cross-core edges, no collectives).

Per-graph dense-scatter formulation (no per-edge DMA gather: the baseline's
gpsimd SWDGE descriptor generation was 98% of exec time):
    S_k[n, m] = sum_{e: src=n, tgt=m} X[e, k]     (100x100 per graph, k=0,1)
    out_g     = sum_k (S_k @ h_g) @ W_k + bias

Device pipeline per graph (1600 edges padded to 13 blocks of 128):
  1. DVE one-hot: O[e, b, m] = (tgt[e,b] == iota[m])          [128, 13, 128] bf16
  2. DVE:        Xall[e, b, k, s] = X[e, b, k] * (e//16 == s) [128, 13, 2, 8]
  3. PE S-build: psS[m, (b,k,s)] += O_b^T @ Xall_b  -> S^T    [128, 208] (13 mm)
  4. PE stage A: psMT = h_g^T-free @ S^T   (lhsT=h_g [m,c])   [128 c, 208]
  5. PE stage B: psO[n, o] = sum_k mT_k^T @ W_k               [104, 128]
  6. DVE bias add from PSUM -> staging; batched DMA out (5 graphs).

All matmul operands bf16 (FWL weight loads), PSUM f32, output f32.
"""

import numpy as np

B, NPG, DEG, K, CIN, COUT = 1000, 100, 16, 2, 128, 128
E = B * NPG * DEG            # 1,600,000 edges
NT = B * NPG                 # 100,000 nodes
NCORES = 8
G_C = B // NCORES            # 125 graphs / core
NT_C = NT // NCORES          # 12,500 nodes / core
E_C = E // NCORES            # 200,000 edges / core
EG = NPG * DEG               # 1600 edges / graph
BLK = 13                     # 128-edge blocks per graph (1664 padded)
BLKP = 16                    # tgt cols padded for 4B-aligned per-graph slices
EGP = BLK * 128              # 1664 padded edges / graph
NSEG = BLK * 8               # 104 src slots per graph (100 real)
SCOL = BLK * K * 8           # 208 S^T columns (b, s, k)
OUTB = 5                     # graphs per output DMA batch

_module_cache = {}


def _patch_tile_drain():
    """This walrus build allows a single sync-wait per instruction; Tile's
    kernel-tail drain aggregates one wait per outstanding sem onto one
    InstDrain. Hoist extras onto dedicated sync nops (sequential on SP)."""
    import concourse.mybir as mybir
    from concourse.tile import TileContext
    from concourse.vector_clock import ScopedClock

    if getattr(TileContext, "_drain_patched", False):
        return

    def _drain_and_barrier(self, tick_clock, wait_clock):
        probe = self.nc.sync.nop(nofuse=True)
        wait_clock.add_sem_waits(probe.ins, ScopedClock({None: tick_clock.global_clock}))
        si = probe.ins.sync_info
        waits = list(si.on_wait) if si is not None and si.on_wait else []
        if si is not None and len(waits) > 1:
            si.on_wait = waits[:1]
            for w in waits[1:]:
                n = self.nc.sync.nop(nofuse=True)
                n.ins.sync_info = mybir.SyncInfo(on_wait=[w], on_update=[])
        self.nc.sync.drain()
        self.nc.all_engine_barrier()
        assert self.sems is not None
        popped = self.nc._tile_sem_poison_stack.pop()
        assert popped is self._sem_poison
        self.nc.clear_and_free_semaphores(list(self.sems.allocated().values()))
        self.nc.all_engine_barrier()

    TileContext._drain_and_barrier = _drain_and_barrier
    TileContext._drain_patched = True


def _build_module():
    import concourse.bacc as bacc
    import concourse.mybir as mybir
    from concourse.tile import TileContext

    _patch_tile_drain()
    f32 = mybir.dt.float32
    bf16 = mybir.dt.bfloat16

    nc = bacc.Bacc("TRN2", target_bir_lowering=False)
    # all inputs partition-major from host
    h_t = nc.dram_tensor("h", [128, G_C * CIN], bf16, kind="ExternalInput")
    tgt_t = nc.dram_tensor("tgt", [128, G_C * BLKP], bf16, kind="ExternalInput")
    xr_t = nc.dram_tensor("xr", [128, G_C * BLK * K], bf16, kind="ExternalInput")
    # iota2[p, m, b] = m: materialized so the innermost (b) dim is stride-1
    # for every DVE operand -> 2x 16-bit mode stays enabled
    iota_t = nc.dram_tensor("iota", [128, NPG * BLK], bf16, kind="ExternalInput")
    mask_t = nc.dram_tensor("mask", [128, 8 * K], bf16, kind="ExternalInput")
    w_t = nc.dram_tensor("w", [128, K * COUT], bf16, kind="ExternalInput")
    bias_t = nc.dram_tensor("bias", [128, COUT], f32, kind="ExternalInput")
    out_t = nc.dram_tensor("out", [NT_C, COUT], f32, kind="ExternalOutput")

    with TileContext(nc) as tc:
        with (
            tc.tile_pool(name="consts", bufs=1) as cpool,
            tc.tile_pool(name="op", bufs=2) as opool,
            tc.tile_pool(name="xap", bufs=2) as xapool,
            tc.tile_pool(name="stp", bufs=2) as stpool,
            tc.tile_pool(name="mtp", bufs=2) as mtpool,
            tc.tile_pool(name="outp", bufs=2) as outpool,
            tc.tile_pool(name="psS", bufs=2, space="PSUM") as psumS,
            tc.tile_pool(name="psMT", bufs=2, space="PSUM") as psumMT,
            tc.tile_pool(name="psO", bufs=2, space="PSUM") as psumO,
        ):
            h_sb = cpool.tile([128, G_C, CIN], bf16)
            nc.sync.dma_start(h_sb[:, :, :],
                              h_t[:, :].rearrange("p (g c) -> p g c", c=CIN))
            tgt_sb = cpool.tile([128, G_C, BLKP], bf16)
            nc.sync.dma_start(tgt_sb[:, :, :],
                              tgt_t[:, :].rearrange("p (g b) -> p g b", b=BLKP))
            xr_sb = cpool.tile([128, G_C, BLK, K], bf16)
            nc.sync.dma_start(xr_sb[:, :, :, :],
                              xr_t[:, :].rearrange("p (g b k) -> p g b k",
                                                   b=BLK, k=K))
            iota_sb = cpool.tile([128, NPG, BLK], bf16)
            nc.sync.dma_start(iota_sb[:, :, :],
                              iota_t[:, :].rearrange("p (m b) -> p m b", b=BLK))
            mask_sb = cpool.tile([128, 8, K], bf16)
            nc.sync.dma_start(mask_sb[:, :, :],
                              mask_t[:, :].rearrange("p (s k) -> p s k", k=K))
            w_sb = cpool.tile([128, K, COUT], bf16)
            nc.sync.dma_start(w_sb[:, :, :],
                              w_t[:, :].rearrange("p (k o) -> p k o", o=COUT))
            bias_sb = cpool.tile([128, COUT], f32)
            nc.sync.dma_start(bias_sb[:, :], bias_t[:, :])

            for g in range(G_C):
                # one-hot of local tgt: O[e, m, b] = (tgt[e, b] == m)
                # [p, m, b] layout keeps innermost dims stride-1 on all APs
                o_tl = opool.tile([128, NPG, BLK], bf16)
                nc.vector.tensor_tensor(
                    o_tl[:, :, :],
                    iota_sb[:, :, :],
                    tgt_sb[:, g, 0:BLK].unsqueeze(1)
                    .broadcast_to([128, NPG, BLK]),
                    op=mybir.AluOpType.is_equal,
                )
                # block-diag X: Xall[e, b, s, k] = X[e, b, k] * (e//16 == s)
                xall_tl = xapool.tile([128, BLK, 8 * K], bf16)
                nc.gpsimd.tensor_tensor(
                    xall_tl[:, :, :].rearrange("p b (s k) -> p b s k", k=K),
                    xr_sb[:, g, :, :].unsqueeze(2).broadcast_to([128, BLK, 8, K]),
                    mask_sb[:, :, :].unsqueeze(1).broadcast_to([128, BLK, 8, K]),
                    op=mybir.AluOpType.mult,
                )

                # S^T build: psS[m, (b, s, k)] = O_b^T @ Xall_b
                psS_tl = psumS.tile([128, BLK, K * 8], f32)
                for b in range(BLK):
                    nc.tensor.matmul(
                        psS_tl[0:NPG, b, :],
                        o_tl[:, :, b],          # lhsT [128 e, 100 m]
                        xall_tl[:, b, :],       # rhs  [128 e, 16]
                        start=True, stop=True,
                    )
                sT_tl = stpool.tile([128, SCOL], bf16)
                nc.any.tensor_copy(sT_tl[0:NPG, :],
                                   psS_tl[0:NPG, :, :].rearrange("p b j -> p (b j)"))

                # stage A: psMT[c, (b,s,k)] = sum_m h[m, c] * S^T[m, (b,s,k)]
                psMT_tl = psumMT.tile([128, SCOL], f32)
                nc.tensor.matmul(
                    psMT_tl[:, :],
                    h_sb[0:NPG, g, :],          # lhsT [100 m, 128 c]
                    sT_tl[0:NPG, :],            # rhs  [100 m, 208]
                    start=True, stop=True,
                )
                # copy reorders (b, s, k) -> (k, b, s) so each mT_k slice is
                # contiguous (matmul weight APs allow only one free dim)
                mT_tl = mtpool.tile([128, SCOL], bf16)
                nc.any.tensor_copy(
                    mT_tl[:, :].rearrange("p (k b s) -> p b s k", b=BLK, s=8),
                    psMT_tl[:, :].rearrange("p (b s k) -> p b s k", s=8, k=K))

                # stage B: psO[n, o] = sum_k mT_k^T @ W_k
                psO_tl = psumO.tile([128, COUT], f32)
                for k in range(K):
                    nc.tensor.matmul(
                        psO_tl[0:NSEG, :],
                        mT_tl[:, k * NSEG:(k + 1) * NSEG],  # lhsT [128 c, 104 n]
                        w_sb[:, k, :],          # rhs  [128 c, 128 o]
                        start=(k == 0), stop=(k == K - 1),
                    )

                # bias add + staged output (OUTB graphs per DMA)
                ob = g % OUTB
                if ob == 0:
                    out_tl = outpool.tile([128, OUTB, COUT], f32)
                nc.vector.tensor_tensor(
                    out_tl[0:NPG, ob, :], psO_tl[0:NPG, :], bias_sb[0:NPG, :],
                    op=mybir.AluOpType.add,
                )
                if ob == OUTB - 1:
                    g0 = g - (OUTB - 1)
                    dst = out_t[g0 * NPG:(g + 1) * NPG, :].rearrange(
                        "(gi m) o -> m gi o", gi=OUTB)
                    nc.sync.dma_start(dst, out_tl[0:NPG, :, :])
    nc.compile()
    return nc


def _get_module():
    if "nc" not in _module_cache:
        _module_cache["nc"] = _build_module()
    return _module_cache["nc"]


def _prep_inputs(h, X, tgt, weight, bias):
    """Host-side sharding/layout (indexing + dtype formatting only)."""
    import ml_dtypes
    bf16 = ml_dtypes.bfloat16

    # local tgt ids within each graph's 100-node window
    g_edge = np.arange(E, dtype=np.int64) // EG
    tgt_loc = tgt - g_edge * NPG
    assert tgt_loc.min() >= 0 and tgt_loc.max() < NPG, "tgt escapes graph block"

    # h: [core][p, g, c] bf16, graph rows padded 100 -> 128 (pad rows unused:
    # S^T rows 100..127 are zero because tgt < 100)
    h_pad = np.zeros((NCORES, G_C, 128, CIN), np.float32)
    h_pad[:, :, :NPG, :] = h.reshape(NCORES, G_C, NPG, CIN)
    h_plc = np.ascontiguousarray(h_pad.transpose(0, 2, 1, 3)).reshape(
        NCORES, 128, G_C * CIN).astype(bf16)

    # per-graph edge stream padded 1600 -> 1664 (pad X = 0 kills contribution)
    # tgt block cols padded 13 -> 16 so per-graph slices stay 4B-aligned
    tgt_p = np.zeros((NCORES, G_C, EGP), np.float32)
    tgt_p[:, :, :EG] = tgt_loc.reshape(NCORES, G_C, EG)
    # [core, g, b, p] -> [core, p, g, b]
    tgt_pad = np.zeros((NCORES, 128, G_C, BLKP), np.float32)
    tgt_pad[:, :, :, :BLK] = tgt_p.reshape(
        NCORES, G_C, BLK, 128).transpose(0, 3, 1, 2)
    tgt_plc = np.ascontiguousarray(tgt_pad).reshape(
        NCORES, 128, G_C * BLKP).astype(bf16)

    x_p = np.zeros((NCORES, G_C, EGP, K), np.float32)
    x_p[:, :, :EG, :] = X.reshape(NCORES, G_C, EG, K)
    # [core, g, b, p, k] -> [core, p, g, b, k]
    xr_plc = np.ascontiguousarray(
        x_p.reshape(NCORES, G_C, BLK, 128, K).transpose(0, 3, 1, 2, 4)).reshape(
        NCORES, 128, G_C * BLK * K).astype(bf16)

    # iota2[p, m, b] = m (materialized; innermost b dim is real memory)
    iota = np.broadcast_to(np.arange(NPG, dtype=np.float32)[None, :, None],
                           (128, NPG, BLK))
    iota = np.ascontiguousarray(iota).reshape(128, NPG * BLK).astype(bf16)
    # mask2[p, s, k] = (p//16 == s)
    mask = (np.arange(128)[:, None, None] // 16 == np.arange(8)[None, :, None])
    mask = np.broadcast_to(mask, (128, 8, K)).astype(np.float32)
    mask = np.ascontiguousarray(mask).reshape(128, 8 * K).astype(bf16)

    # W: [c, k, o] bf16 partition-major
    w_plc = np.ascontiguousarray(weight.transpose(1, 0, 2)).reshape(
        128, K * COUT).astype(bf16)
    bias_rep = np.ascontiguousarray(
        np.broadcast_to(bias, (128, COUT))).astype(np.float32)
    return h_plc, tgt_plc, xr_plc, iota, mask, w_plc, bias_rep


def kernel(h, X, edge_index, node_index, batch_node, batch_edge, num_node,
           weight, bias):
    from concourse.bass_utils import run_bass_kernel_spmd

    h = np.asarray(h, np.float32)
    X = np.asarray(X, np.float32)
    edge_index = np.asarray(edge_index)
    weight = np.asarray(weight, np.float32)
    bias = np.asarray(bias, np.float32)

    src = np.asarray(edge_index[1])
    tgt = np.asarray(edge_index[2])
    # structural contract from setup_inputs (see module docstring)
    assert src.shape == (E,) and h.shape == (NT, CIN) and X.shape == (E, K)
    assert np.array_equal(src, np.arange(E, dtype=src.dtype) // DEG), \
        "edges not sorted as src=e//DEG"

    h_plc, tgt_plc, xr_plc, iota, mask, w_plc, bias_rep = _prep_inputs(
        h, X, tgt, weight, bias)

    nc = _get_module()
    in_maps = []
    for c in range(NCORES):
        in_maps.append({
            "h": h_plc[c],
            "tgt": tgt_plc[c],
            "xr": xr_plc[c],
            "iota": iota,
            "mask": mask,
            "w": w_plc,
            "bias": bias_rep,
        })
    res = run_bass_kernel_spmd(nc, in_maps, core_ids=list(range(NCORES)))
    out = np.concatenate([r["out"] for r in res.results], axis=0)
    return out
